# revision 22
# baseline (speedup 1.0000x reference)
"""Multi-head attention kernel for 8 Trainium2 NeuronCores (Bass/Tile).

Problem: B=2, L=2048, D=1024, H=16 heads, DK=64.
Sharding: core c -> batch b = c//4, head-group g = c%4 (4 heads each).
Each core computes its 4 heads' attention + its slice of the output
projection; the host sums the 4 partial outputs per batch (exact, since
Out = sum_g C_g @ Wo_g) and adds the bo / bv-derived bias terms.

Layout strategy (everything flows transposed so no on-chip transposes
are ever needed):
  - host supplies X^T [D, L] per input (bf16)
  - Q^T, K^T [256, L] produced directly (lhsT = W natural, rhs = X^T)
  - S^T[j,i] tiles via lhsT=K^T, rhs=Q^T; two heads packed per matmul
    round via PE row-tiling (K=64 -> row positions 0 and 64)
  - P~ = exp(S^T * scale) on ACT, PSUM -> SBUF (no max subtraction:
    |S*scale| <= ~3 for these input stats)
  - C~^T[dk,i] += V[j,dk].T-free matmul with a concurrent M=1 ones
    matmul (col position 64) accumulating the softmax denominators
  - normalization after the fact: recip(sums) broadcast across 64
    partitions with a K=1 matmul, then one DVE multiply
  - O[i,:] via lhsT=C^T, rhs=Wo natural; DMA out per 128-row tile
"""

import sys

sys.path.insert(0, "/opt/trn_rl_repo")

from contextlib import ExitStack

import ml_dtypes
import numpy as np

import concourse.bass as bass
import concourse.tile as tile
from concourse import bacc, mybir
from concourse.bass_utils import run_bass_kernel_spmd


def _install_ntff_hook_shim():
    """The agent image's ``antenv`` lacks ``axon_hooks``, so the boot shim
    silently skips NTFF-profile-hook registration and ``run_bass_kernel_spmd``
    crashes on import when BASS_TRACE=1. Provide the module and register the
    ctypes hook ourselves; degrade to no-tracing on any failure."""
    import types

    if "antenv.axon_hooks" in sys.modules:
        return
    mod = types.ModuleType("antenv.axon_hooks")
    mod._hook = None
    mod.set_axon_ntff_profile_hook = lambda h: setattr(mod, "_hook", h)
    mod.get_axon_ntff_profile_hook = lambda: mod._hook
    sys.modules["antenv.axon_hooks"] = mod
    try:
        import antenv

        antenv.axon_hooks = mod
    except Exception:
        pass
    try:
        from trn_agent_boot.trn_boot import _ntff_profile_via_ctypes

        mod._hook = _ntff_profile_via_ctypes("/opt/axon/libaxon_pjrt.so")
    except Exception:
        pass


_install_ntff_hook_shim()

B, L, D, H, DK = 2, 2048, 1024, 16, 64
NCORES = 8
GROUPS = 4  # head-groups == cores per batch
NH = H // GROUPS  # 4 heads per core
CG = NH * DK  # 256 projected features per core
DT = D // 128  # 8 contraction tiles
CT = CG // 128  # 2 c-tiles
IT = L // 512  # 4 query blocks of 512
LT = L // 128  # 16 key/query tiles of 128
SCALE = 1.0 / float(np.sqrt(DK))

F32 = mybir.dt.float32
BF16 = mybir.dt.bfloat16
Identity = mybir.ActivationFunctionType.Identity
Exp = mybir.ActivationFunctionType.Exp

_built = None
_last_results = None


import os

_DEBUG = bool(os.environ.get("KERNEL_DEBUG"))


def _build():
    nc = bacc.Bacc()

    xq_d = nc.dram_tensor("xq_t", [D, L], BF16, kind="ExternalInput")
    xk_d = nc.dram_tensor("xk_t", [D, L], BF16, kind="ExternalInput")
    xv_d = nc.dram_tensor("xv_t", [D, L], BF16, kind="ExternalInput")
    wq_d = nc.dram_tensor("wq", [D, CG], BF16, kind="ExternalInput")
    wk_d = nc.dram_tensor("wk", [D, CG], BF16, kind="ExternalInput")
    wv_d = nc.dram_tensor("wv", [D, CG], BF16, kind="ExternalInput")
    wo_d = nc.dram_tensor("wo", [CG, D], F32, kind="ExternalInput")
    bq_d = nc.dram_tensor("bq", [CG], F32, kind="ExternalInput")
    bk_d = nc.dram_tensor("bk", [CG], F32, kind="ExternalInput")
    out_d = nc.dram_tensor("out_p", [L, D], F32, kind="ExternalOutput")
    if _DEBUG:
        dbg = {
            "qT": nc.dram_tensor("d_qT", [CT, 128, L], F32, kind="ExternalOutput"),
            "kT": nc.dram_tensor("d_kT", [CT, 128, L], F32, kind="ExternalOutput"),
            "v": nc.dram_tensor("d_v", [128, LT * NH * 65], F32, kind="ExternalOutput"),
            "craw": nc.dram_tensor("d_craw", [CT, 128, L], F32, kind="ExternalOutput"),
            "sums": nc.dram_tensor("d_sums", [IT, 2048], F32, kind="ExternalOutput"),
            "strip": nc.dram_tensor("d_strip", [97, L], F32, kind="ExternalOutput"),
            "cnorm": nc.dram_tensor("d_cnorm", [CT, 128, L], F32, kind="ExternalOutput"),
        }

    with ExitStack() as ctx:
        tc = ctx.enter_context(tile.TileContext(nc))
        const = ctx.enter_context(tc.tile_pool(name="const", bufs=1))
        xp = ctx.enter_context(tc.tile_pool(name="xp", bufs=2))
        wp = ctx.enter_context(tc.tile_pool(name="wp", bufs=1))
        proj = ctx.enter_context(tc.tile_pool(name="proj", bufs=1))
        pp = ctx.enter_context(tc.tile_pool(name="pp", bufs=3))
        op_ = ctx.enter_context(tc.tile_pool(name="op", bufs=2))
        st = ctx.enter_context(tc.tile_pool(name="st", bufs=2))
        ph1 = ExitStack()
        ps1 = ph1.enter_context(tc.tile_pool(name="ps1", bufs=2, space="PSUM"))

        ones64 = const.tile([128, 64], F32)
        nc.vector.memset(ones64, 1.0)

        # ---------------- weights + biases ----------------
        wq_sb = wp.tile([128, DT, CG], BF16, tag="wq")
        wk_sb = wp.tile([128, DT, CG], BF16, tag="wk")
        wv_sb = wp.tile([128, DT, CG], BF16, tag="wv")
        wo_sb = wp.tile([128, CT, D], F32, tag="wo")
        nc.sync.dma_start(out=wq_sb, in_=wq_d[:, :].rearrange("(t p) c -> p t c", p=128))
        nc.sync.dma_start(out=wk_sb, in_=wk_d[:, :].rearrange("(t p) c -> p t c", p=128))
        nc.sync.dma_start(out=wv_sb, in_=wv_d[:, :].rearrange("(t p) c -> p t c", p=128))
        nc.sync.dma_start(out=wo_sb, in_=wo_d[:, :].rearrange("(t p) d -> p t d", p=128))
        bq_sb = wp.tile([128, CT], F32, tag="bq")
        bk_sb = wp.tile([128, CT], F32, tag="bk")
        nc.sync.dma_start(out=bq_sb, in_=bq_d[:].rearrange("(c p) -> p c", p=128))
        nc.sync.dma_start(out=bk_sb, in_=bk_d[:].rearrange("(c p) -> p c", p=128))

        # ---------------- phase 1: projections ----------------
        # V first (attention's C-matmuls need it), then K^T, then Q^T.
        with nc.named_scope("proj_v"):
            xv_sb = xp.tile([128, DT, L], BF16, tag="x", name="xv_sb")
            nc.sync.dma_start(
                out=xv_sb, in_=xv_d[:, :].rearrange("(t p) l -> p t l", p=128)
            )
            # v_sb holds [V_h | 1] blocks of 65 columns per head: the ones
            # column makes the C~ matmul also accumulate the softmax
            # denominator in psum row 64 (M=65 costs the same as M=64).
            v_sb = proj.tile([128, LT, NH * 65], F32, tag="v")
            nc.vector.memset(v_sb, 1.0)
            for lt in range(LT):
                v_ps = ps1.tile([128, CG], F32, tag="vps")
                for dt in range(DT):
                    nc.tensor.matmul(
                        v_ps,
                        lhsT=xv_sb[:, dt, lt * 128 : (lt + 1) * 128],
                        rhs=wv_sb[:, dt, :],
                        start=(dt == 0),
                        stop=(dt == DT - 1),
                    )
                # bv is folded into the host-side output bias (exact for
                # normalized softmax), so a plain copy suffices here.
                for h in range(NH):
                    nc.vector.tensor_copy(
                        out=v_sb[:, lt, 65 * h : 65 * h + 64],
                        in_=v_ps[:, 64 * h : 64 * h + 64],
                    )

        def project_T(x_sb, w_sb, b_sb, out_tiles, scope):
            with nc.named_scope(scope):
                for ct in range(CT):
                    for it in range(IT):
                        t_ps = ps1.tile([128, 512], F32, tag="tps", name="t_ps")
                        for dt in range(DT):
                            nc.tensor.matmul(
                                t_ps,
                                lhsT=w_sb[:, dt, ct * 128 : (ct + 1) * 128],
                                rhs=x_sb[:, dt, it * 512 : (it + 1) * 512],
                                start=(dt == 0),
                                stop=(dt == DT - 1),
                            )
                        nc.scalar.activation(
                            out=out_tiles[ct][:, it * 512 : (it + 1) * 512],
                            in_=t_ps,
                            func=Identity,
                            bias=b_sb[:, ct : ct + 1],
                            scale=1.0,
                        )

        xk_sb = xp.tile([128, DT, L], BF16, tag="x", name="xk_sb")
        nc.sync.dma_start(out=xk_sb, in_=xk_d[:, :].rearrange("(t p) l -> p t l", p=128))
        kT = [proj.tile([128, L], F32, tag=f"kT{ct}", name=f"kT{ct}") for ct in range(CT)]
        project_T(xk_sb, wk_sb, bk_sb, kT, "proj_k")

        xq_sb = xp.tile([128, DT, L], BF16, tag="x", name="xq_sb")
        nc.sync.dma_start(out=xq_sb, in_=xq_d[:, :].rearrange("(t p) l -> p t l", p=128))
        qT = [proj.tile([128, L], F32, tag=f"qT{ct}", name=f"qT{ct}") for ct in range(CT)]
        project_T(xq_sb, wq_sb, bq_sb, qT, "proj_q")

        if _DEBUG:
            for ct in range(CT):
                nc.sync.dma_start(out=dbg["qT"][ct], in_=qT[ct])
                nc.sync.dma_start(out=dbg["kT"][ct], in_=kT[ct])
            nc.sync.dma_start(
                out=dbg["v"][:, :], in_=v_sb[:, :, :].rearrange("p a b -> p (a b)")
            )

        # ---------------- phase 2+3: attention, streamed per i-block ----------------
        ph1.close()  # release phase-1 PSUM banks
        pss = ctx.enter_context(tc.tile_pool(name="pss", bufs=2, space="PSUM"))
        psc = ctx.enter_context(tc.tile_pool(name="psc", bufs=2, space="PSUM"))
        psn = ctx.enter_context(tc.tile_pool(name="psn", bufs=1, space="PSUM"))
        pso = ctx.enter_context(tc.tile_pool(name="pso", bufs=1, space="PSUM"))
        cT = [proj.tile([128, L], F32, tag=f"cT{ct}", name=f"cT{ct}") for ct in range(CT)]
        strip = st.tile([97, L], F32, tag="strip", bufs=1)

        for it in range(IT):
            isl = slice(it * 512, (it + 1) * 512)
            stage = st.tile([65, 2048], F32, tag="stage")
            with nc.named_scope(f"attn_i{it}"):
                for hp in range(2):
                    cps = [
                        psc.tile([65, 512], F32, tag="cps", name=f"cps{hl}")
                        for hl in range(2)
                    ]
                    for jt in range(LT):
                        s_ps = pss.tile([128, 1024], F32, tag="sps", name="s_ps")
                        for hl in range(2):
                            # heads 2*hp+hl; rows 64*hl of the hp-th 128-row tile
                            rsl = slice(64 * hl, 64 * hl + 64)
                            nc.tensor.matmul(
                                s_ps[:, hl * 512 : (hl + 1) * 512],
                                lhsT=kT[hp][rsl, jt * 128 : (jt + 1) * 128],
                                rhs=qT[hp][rsl, isl],
                                start=True,
                                stop=True,
                            )
                        p_t = pp.tile([128, 1024], F32, tag="pt", name="p_t")
                        nc.scalar.activation(out=p_t, in_=s_ps, func=Exp, scale=SCALE)
                        for hl in range(2):
                            h = 2 * hp + hl
                            psl = slice(hl * 512, (hl + 1) * 512)
                            nc.tensor.matmul(
                                cps[hl],
                                lhsT=v_sb[:, jt, 65 * h : 65 * h + 65],
                                rhs=p_t[:, psl],
                                start=(jt == 0),
                                stop=(jt == LT - 1),
                            )
                    for hl in range(2):
                        h = 2 * hp + hl
                        nc.vector.tensor_copy(
                            out=cT[hp][64 * hl : 64 * hl + 64, isl],
                            in_=cps[hl][0:64, :],
                        )
                        nc.vector.tensor_copy(
                            out=stage[64:65, h * 512 : (h + 1) * 512],
                            in_=cps[hl][64:65, :],
                        )

            if _DEBUG:
                for ct in range(CT):
                    nc.sync.dma_start(out=dbg["craw"][ct][:, isl], in_=cT[ct][:, isl])
                nc.sync.dma_start(
                    out=dbg["sums"][it : it + 1, :], in_=stage[64:65, :]
                )

            with nc.named_scope(f"norm_o_i{it}"):
                # reciprocal of the four heads' denominators (1 lane; overlaps
                # with the next i-block's ACT-bound attention stream).
                # reciprocal_approx_fast is a silent no-op on this HW for a
                # single-partition in-place AP, so use the exact iterative one.
                nc.vector.reciprocal(out=stage[64:65, :], in_=stage[64:65, :])
                for h in range(NH):
                    nc.sync.dma_start(
                        out=strip[32 * h : 32 * h + 1, isl],
                        in_=stage[64:65, h * 512 : (h + 1) * 512],
                    )
                for hp in range(2):
                    n_ps = psn.tile([128, 512], F32, tag="nps")
                    for hl in range(2):
                        h = 2 * hp + hl
                        nc.tensor.matmul(
                            n_ps[64 * hl : 64 * hl + 64, :],
                            lhsT=ones64[32 * h : 32 * h + 1, :],
                            rhs=strip[32 * h : 32 * h + 1, isl],
                            start=True,
                            stop=True,
                            tile_position=(32 * h, 64 * hl),
                        )
                    for hl in range(2):
                        rsl = slice(64 * hl, 64 * hl + 64)
                        nc.vector.tensor_mul(
                            out=cT[hp][rsl, isl],
                            in0=cT[hp][rsl, isl],
                            in1=n_ps[rsl, :],
                        )
                if _DEBUG:
                    for ct in range(CT):
                        nc.sync.dma_start(
                            out=dbg["cnorm"][ct][:, isl], in_=cT[ct][:, isl]
                        )
                    if it == IT - 1:
                        nc.sync.dma_start(out=dbg["strip"][:, :], in_=strip)
                for s in range(4):
                    i0 = it * 512 + s * 128
                    o_sb = op_.tile([128, D], F32, tag="osb")
                    for dn in range(2):
                        o_ps = pso.tile([128, 512], F32, tag="ops")
                        for ct in range(CT):
                            nc.tensor.matmul(
                                o_ps,
                                lhsT=cT[ct][:, i0 : i0 + 128],
                                rhs=wo_sb[:, ct, dn * 512 : (dn + 1) * 512],
                                start=(ct == 0),
                                stop=(ct == CT - 1),
                            )
                        nc.vector.tensor_copy(
                            out=o_sb[:, dn * 512 : (dn + 1) * 512], in_=o_ps
                        )
                    nc.sync.dma_start(out=out_d[i0 : i0 + 128, :], in_=o_sb)

    nc.compile()
    return nc


def _get_built():
    global _built
    if _built is None:
        _built = _build()
    return _built


def _make_in_maps(query, key, value, Wq, bq, Wk, bk, Wv, bv, Wo, bo):
    bf = ml_dtypes.bfloat16
    xt = {}
    for b in range(B):
        xt[b] = {
            "xq_t": np.ascontiguousarray(query[b].T).astype(bf),
            "xk_t": np.ascontiguousarray(key[b].T).astype(bf),
            "xv_t": np.ascontiguousarray(value[b].T).astype(bf),
        }
    in_maps = []
    for c in range(NCORES):
        b, g = c // GROUPS, c % GROUPS
        cols = slice(g * CG, (g + 1) * CG)
        in_maps.append(
            {
                **xt[b],
                "wq": np.ascontiguousarray(Wq[:, cols]).astype(bf),
                "wk": np.ascontiguousarray(Wk[:, cols]).astype(bf),
                "wv": np.ascontiguousarray(Wv[:, cols]).astype(bf),
                "wo": np.ascontiguousarray(Wo[cols, :], dtype=np.float32),
                "bq": np.ascontiguousarray(bq[cols], dtype=np.float32),
                "bk": np.ascontiguousarray(bk[cols], dtype=np.float32),
            }
        )
    return in_maps


def kernel(query, key, value, Wq, bq, Wk, bk, Wv, bv, Wo, bo):
    global _last_results
    query = np.asarray(query, dtype=np.float32)
    key = np.asarray(key, dtype=np.float32)
    value = np.asarray(value, dtype=np.float32)
    Wq, Wk, Wv, Wo = (np.asarray(w, dtype=np.float32) for w in (Wq, Wk, Wv, Wo))
    bq, bk, bv, bo = (np.asarray(v, dtype=np.float32) for v in (bq, bk, bv, bo))

    nc = _get_built()
    in_maps = _make_in_maps(query, key, value, Wq, bq, Wk, bk, Wv, bv, Wo, bo)
    res = run_bass_kernel_spmd(nc, in_maps, core_ids=list(range(NCORES)))
    _last_results = res

    # bv contributes exactly bv @ Wo to every output row (softmax rows sum
    # to 1); bo is the plain output bias.
    bias = (bv @ Wo + bo).astype(np.float32)
    out = np.empty((B, L, D), dtype=np.float32)
    for b in range(B):
        acc = np.zeros((L, D), dtype=np.float32)
        for g in range(GROUPS):
            acc += res.results[b * GROUPS + g]["out_p"]
        out[b] = acc + bias
    return out


# revision 24
# speedup vs baseline: 1.0746x; 1.0746x over previous
"""Multi-head attention kernel for 8 Trainium2 NeuronCores (Bass/Tile).

Problem: B=2, L=2048, D=1024, H=16 heads, DK=64.
Sharding: core c -> batch b = c//4, head-group g = c%4 (4 heads each).
Each core computes its 4 heads' attention + its slice of the output
projection; the host sums the 4 partial outputs per batch (exact, since
Out = sum_g C_g @ Wo_g) and adds the bo / bv-derived bias terms.

Layout strategy (everything flows transposed so no on-chip transposes
are ever needed):
  - host supplies X^T [D, L] per input (bf16)
  - Q^T, K^T [256, L] produced directly (lhsT = W natural, rhs = X^T)
  - S^T[j,i] tiles via lhsT=K^T, rhs=Q^T; two heads packed per matmul
    round via PE row-tiling (K=64 -> row positions 0 and 64)
  - P~ = exp(S^T * scale) on ACT, PSUM -> SBUF (no max subtraction:
    |S*scale| <= ~3 for these input stats)
  - C~^T[dk,i] += V[j,dk].T-free matmul with a concurrent M=1 ones
    matmul (col position 64) accumulating the softmax denominators
  - normalization after the fact: recip(sums) broadcast across 64
    partitions with a K=1 matmul, then one DVE multiply
  - O[i,:] via lhsT=C^T, rhs=Wo natural; DMA out per 128-row tile
"""

import sys

sys.path.insert(0, "/opt/trn_rl_repo")

from contextlib import ExitStack

import ml_dtypes
import numpy as np

import concourse.bass as bass
import concourse.tile as tile
from concourse import bacc, mybir
from concourse.bass_utils import run_bass_kernel_spmd


def _install_ntff_hook_shim():
    """The agent image's ``antenv`` lacks ``axon_hooks``, so the boot shim
    silently skips NTFF-profile-hook registration and ``run_bass_kernel_spmd``
    crashes on import when BASS_TRACE=1. Provide the module and register the
    ctypes hook ourselves; degrade to no-tracing on any failure."""
    import types

    if "antenv.axon_hooks" in sys.modules:
        return
    mod = types.ModuleType("antenv.axon_hooks")
    mod._hook = None
    mod.set_axon_ntff_profile_hook = lambda h: setattr(mod, "_hook", h)
    mod.get_axon_ntff_profile_hook = lambda: mod._hook
    sys.modules["antenv.axon_hooks"] = mod
    try:
        import antenv

        antenv.axon_hooks = mod
    except Exception:
        pass
    try:
        from trn_agent_boot.trn_boot import _ntff_profile_via_ctypes

        mod._hook = _ntff_profile_via_ctypes("/opt/axon/libaxon_pjrt.so")
    except Exception:
        pass


_install_ntff_hook_shim()

B, L, D, H, DK = 2, 2048, 1024, 16, 64
NCORES = 8
GROUPS = 4  # head-groups == cores per batch
NH = H // GROUPS  # 4 heads per core
CG = NH * DK  # 256 projected features per core
DT = D // 128  # 8 contraction tiles
CT = CG // 128  # 2 c-tiles
IT = L // 512  # 4 query blocks of 512
LT = L // 128  # 16 key/query tiles of 128
SCALE = 1.0 / float(np.sqrt(DK))

F32 = mybir.dt.float32
BF16 = mybir.dt.bfloat16
Identity = mybir.ActivationFunctionType.Identity
Exp = mybir.ActivationFunctionType.Exp

_built = None
_last_results = None


import os

_DEBUG = bool(os.environ.get("KERNEL_DEBUG"))

if os.environ.get("KERNEL_LDW_OPT"):
    # A/B experiment: let walrus run its LDWEIGHTS optimization.
    from concourse import bass_utils as _bu

    _orig_run_command = _bu.run_command

    def _patched_run_command(cmd, *a, **kw):
        cmd = [
            c.replace("--enable-ldw-opt=false", "--enable-ldw-opt=true")
            if isinstance(c, str)
            else c
            for c in cmd
        ]
        return _orig_run_command(cmd, *a, **kw)

    _bu.run_command = _patched_run_command


def _build():
    nc = bacc.Bacc()

    xq_d = nc.dram_tensor("xq_t", [D, L], BF16, kind="ExternalInput")
    xk_d = nc.dram_tensor("xk_t", [D, L], BF16, kind="ExternalInput")
    xv_d = nc.dram_tensor("xv_t", [D, L], BF16, kind="ExternalInput")
    wq_d = nc.dram_tensor("wq", [D, CG], BF16, kind="ExternalInput")
    wk_d = nc.dram_tensor("wk", [D, CG], BF16, kind="ExternalInput")
    wv_d = nc.dram_tensor("wv", [D, CG], BF16, kind="ExternalInput")
    wo_d = nc.dram_tensor("wo", [CG, D], F32, kind="ExternalInput")
    bq_d = nc.dram_tensor("bq", [CG], F32, kind="ExternalInput")
    bk_d = nc.dram_tensor("bk", [CG], F32, kind="ExternalInput")
    out_d = nc.dram_tensor("out_p", [L, D], F32, kind="ExternalOutput")
    if _DEBUG:
        dbg = {
            "qT": nc.dram_tensor("d_qT", [CT, 128, L], F32, kind="ExternalOutput"),
            "kT": nc.dram_tensor("d_kT", [CT, 128, L], F32, kind="ExternalOutput"),
            "v": nc.dram_tensor("d_v", [128, LT * NH * 65], F32, kind="ExternalOutput"),
            "craw": nc.dram_tensor("d_craw", [CT, 128, L], F32, kind="ExternalOutput"),
            "sums": nc.dram_tensor("d_sums", [IT, 2048], F32, kind="ExternalOutput"),
            "strip": nc.dram_tensor("d_strip", [97, L], F32, kind="ExternalOutput"),
            "cnorm": nc.dram_tensor("d_cnorm", [CT, 128, L], F32, kind="ExternalOutput"),
        }

    with ExitStack() as ctx:
        tc = ctx.enter_context(tile.TileContext(nc))
        const = ctx.enter_context(tc.tile_pool(name="const", bufs=1))
        xp = ctx.enter_context(tc.tile_pool(name="xp", bufs=2))
        wp = ctx.enter_context(tc.tile_pool(name="wp", bufs=1))
        proj = ctx.enter_context(tc.tile_pool(name="proj", bufs=1))
        pp = ctx.enter_context(tc.tile_pool(name="pp", bufs=3))
        op_ = ctx.enter_context(tc.tile_pool(name="op", bufs=2))
        st = ctx.enter_context(tc.tile_pool(name="st", bufs=2))
        ph1 = ExitStack()
        ps1 = ph1.enter_context(tc.tile_pool(name="ps1", bufs=2, space="PSUM"))

        ones64 = const.tile([128, 64], F32)
        nc.vector.memset(ones64, 1.0)

        # ---------------- weights + biases ----------------
        wq_sb = wp.tile([128, DT, CG], BF16, tag="wq")
        wk_sb = wp.tile([128, DT, CG], BF16, tag="wk")
        wv_sb = wp.tile([128, DT, CG], BF16, tag="wv")
        wo_sb = wp.tile([128, CT, D], F32, tag="wo")
        nc.sync.dma_start(out=wq_sb, in_=wq_d[:, :].rearrange("(t p) c -> p t c", p=128))
        nc.sync.dma_start(out=wk_sb, in_=wk_d[:, :].rearrange("(t p) c -> p t c", p=128))
        nc.sync.dma_start(out=wv_sb, in_=wv_d[:, :].rearrange("(t p) c -> p t c", p=128))
        nc.sync.dma_start(out=wo_sb, in_=wo_d[:, :].rearrange("(t p) d -> p t d", p=128))
        bq_sb = wp.tile([128, CT], F32, tag="bq")
        bk_sb = wp.tile([128, CT], F32, tag="bk")
        nc.sync.dma_start(out=bq_sb, in_=bq_d[:].rearrange("(c p) -> p c", p=128))
        nc.sync.dma_start(out=bk_sb, in_=bk_d[:].rearrange("(c p) -> p c", p=128))

        # ---------------- phase 1: projections ----------------
        # V first (attention's C-matmuls need it), then K^T, then Q^T.
        with nc.named_scope("proj_v"):
            xv_sb = xp.tile([128, DT, L], BF16, tag="x", name="xv_sb")
            nc.sync.dma_start(
                out=xv_sb, in_=xv_d[:, :].rearrange("(t p) l -> p t l", p=128)
            )
            # v_sb holds [V_h | 1] blocks of 65 columns per head: the ones
            # column makes the C~ matmul also accumulate the softmax
            # denominator in psum row 64 (M=65 costs the same as M=64).
            v_sb = proj.tile([128, LT, NH * 65], F32, tag="v")
            nc.vector.memset(v_sb, 1.0)
            for lt in range(LT):
                v_ps = ps1.tile([128, CG], F32, tag="vps")
                for dt in range(DT):
                    nc.tensor.matmul(
                        v_ps,
                        lhsT=xv_sb[:, dt, lt * 128 : (lt + 1) * 128],
                        rhs=wv_sb[:, dt, :],
                        start=(dt == 0),
                        stop=(dt == DT - 1),
                    )
                # bv is folded into the host-side output bias (exact for
                # normalized softmax), so a plain copy suffices here.
                for h in range(NH):
                    nc.vector.tensor_copy(
                        out=v_sb[:, lt, 65 * h : 65 * h + 64],
                        in_=v_ps[:, 64 * h : 64 * h + 64],
                    )

        def project_T(x_sb, w_sb, b_sb, out_tiles, scope):
            with nc.named_scope(scope):
                for ct in range(CT):
                    for it in range(IT):
                        t_ps = ps1.tile([128, 512], F32, tag="tps", name="t_ps")
                        for dt in range(DT):
                            nc.tensor.matmul(
                                t_ps,
                                lhsT=w_sb[:, dt, ct * 128 : (ct + 1) * 128],
                                rhs=x_sb[:, dt, it * 512 : (it + 1) * 512],
                                start=(dt == 0),
                                stop=(dt == DT - 1),
                            )
                        nc.scalar.activation(
                            out=out_tiles[ct][:, it * 512 : (it + 1) * 512],
                            in_=t_ps,
                            func=Identity,
                            bias=b_sb[:, ct : ct + 1],
                            scale=1.0,
                        )

        xk_sb = xp.tile([128, DT, L], BF16, tag="x", name="xk_sb")
        nc.sync.dma_start(out=xk_sb, in_=xk_d[:, :].rearrange("(t p) l -> p t l", p=128))
        kT = [proj.tile([128, L], F32, tag=f"kT{ct}", name=f"kT{ct}") for ct in range(CT)]
        project_T(xk_sb, wk_sb, bk_sb, kT, "proj_k")

        xq_sb = xp.tile([128, DT, L], BF16, tag="x", name="xq_sb")
        nc.sync.dma_start(out=xq_sb, in_=xq_d[:, :].rearrange("(t p) l -> p t l", p=128))
        qT = [proj.tile([128, L], F32, tag=f"qT{ct}", name=f"qT{ct}") for ct in range(CT)]
        project_T(xq_sb, wq_sb, bq_sb, qT, "proj_q")

        if _DEBUG:
            for ct in range(CT):
                nc.sync.dma_start(out=dbg["qT"][ct], in_=qT[ct])
                nc.sync.dma_start(out=dbg["kT"][ct], in_=kT[ct])
            nc.sync.dma_start(
                out=dbg["v"][:, :], in_=v_sb[:, :, :].rearrange("p a b -> p (a b)")
            )

        # ---------------- phase 2+3: attention, streamed per i-block ----------------
        ph1.close()  # release phase-1 PSUM banks
        pss = ctx.enter_context(tc.tile_pool(name="pss", bufs=2, space="PSUM"))
        psc = ctx.enter_context(tc.tile_pool(name="psc", bufs=2, space="PSUM"))
        psn = ctx.enter_context(tc.tile_pool(name="psn", bufs=1, space="PSUM"))
        pso = ctx.enter_context(tc.tile_pool(name="pso", bufs=1, space="PSUM"))
        cT = [proj.tile([128, L], F32, tag=f"cT{ct}", name=f"cT{ct}") for ct in range(CT)]
        strip = st.tile([97, L], F32, tag="strip", bufs=1)

        for it in range(IT):
            isl = slice(it * 512, (it + 1) * 512)
            stage = st.tile([65, 2048], F32, tag="stage")
            with nc.named_scope(f"attn_i{it}"):
                for hp in range(2):
                    cps = [
                        psc.tile([65, 512], F32, tag="cps", name=f"cps{hl}")
                        for hl in range(2)
                    ]
                    for jt in range(LT):
                        s_ps = pss.tile([128, 1024], F32, tag="sps", name="s_ps")
                        for hl in range(2):
                            # heads 2*hp+hl; rows 64*hl of the hp-th 128-row tile
                            rsl = slice(64 * hl, 64 * hl + 64)
                            nc.tensor.matmul(
                                s_ps[:, hl * 512 : (hl + 1) * 512],
                                lhsT=kT[hp][rsl, jt * 128 : (jt + 1) * 128],
                                rhs=qT[hp][rsl, isl],
                                start=True,
                                stop=True,
                            )
                        p_t = pp.tile([128, 1024], F32, tag="pt", name="p_t")
                        nc.scalar.activation(out=p_t, in_=s_ps, func=Exp, scale=SCALE)
                        for hl in range(2):
                            h = 2 * hp + hl
                            psl = slice(hl * 512, (hl + 1) * 512)
                            nc.tensor.matmul(
                                cps[hl],
                                lhsT=v_sb[:, jt, 65 * h : 65 * h + 65],
                                rhs=p_t[:, psl],
                                start=(jt == 0),
                                stop=(jt == LT - 1),
                            )
                    for hl in range(2):
                        h = 2 * hp + hl
                        nc.vector.tensor_copy(
                            out=cT[hp][64 * hl : 64 * hl + 64, isl],
                            in_=cps[hl][0:64, :],
                        )
                        nc.vector.tensor_copy(
                            out=stage[64:65, h * 512 : (h + 1) * 512],
                            in_=cps[hl][64:65, :],
                        )

            if _DEBUG:
                for ct in range(CT):
                    nc.sync.dma_start(out=dbg["craw"][ct][:, isl], in_=cT[ct][:, isl])
                nc.sync.dma_start(
                    out=dbg["sums"][it : it + 1, :], in_=stage[64:65, :]
                )

            with nc.named_scope(f"norm_o_i{it}"):
                # Reciprocal of the four heads' denominators. DVE reciprocal
                # is ~8 cyc/elem and partition-parallel, so spread the 2048
                # sums across 128 partitions with a tiny SBUF->SBUF DMA first
                # (a single-row reciprocal would serialize ~17us on one lane).
                sq = st.tile([128, 16], F32, tag="sq")
                sq2 = st.tile([128, 16], F32, tag="sq2")
                nc.sync.dma_start(out=sq[:, :], in_=stage[64:65, :])
                nc.vector.reciprocal(out=sq2, in_=sq)
                for h in range(NH):
                    # sq2 partitions 32h..32h+31 hold head h's 512 values
                    nc.sync.dma_start(
                        out=strip[32 * h : 32 * h + 1, isl],
                        in_=sq2[32 * h : 32 * h + 32, :],
                    )
                for hp in range(2):
                    n_ps = psn.tile([128, 512], F32, tag="nps")
                    for hl in range(2):
                        h = 2 * hp + hl
                        nc.tensor.matmul(
                            n_ps[64 * hl : 64 * hl + 64, :],
                            lhsT=ones64[32 * h : 32 * h + 1, :],
                            rhs=strip[32 * h : 32 * h + 1, isl],
                            start=True,
                            stop=True,
                            tile_position=(32 * h, 64 * hl),
                        )
                    for hl in range(2):
                        rsl = slice(64 * hl, 64 * hl + 64)
                        nc.vector.tensor_mul(
                            out=cT[hp][rsl, isl],
                            in0=cT[hp][rsl, isl],
                            in1=n_ps[rsl, :],
                        )
                if _DEBUG:
                    for ct in range(CT):
                        nc.sync.dma_start(
                            out=dbg["cnorm"][ct][:, isl], in_=cT[ct][:, isl]
                        )
                    if it == IT - 1:
                        nc.sync.dma_start(out=dbg["strip"][:, :], in_=strip)
                for s in range(4):
                    i0 = it * 512 + s * 128
                    o_sb = op_.tile([128, D], F32, tag="osb")
                    for dn in range(2):
                        o_ps = pso.tile([128, 512], F32, tag="ops")
                        for ct in range(CT):
                            nc.tensor.matmul(
                                o_ps,
                                lhsT=cT[ct][:, i0 : i0 + 128],
                                rhs=wo_sb[:, ct, dn * 512 : (dn + 1) * 512],
                                start=(ct == 0),
                                stop=(ct == CT - 1),
                            )
                        nc.vector.tensor_copy(
                            out=o_sb[:, dn * 512 : (dn + 1) * 512], in_=o_ps
                        )
                    nc.sync.dma_start(out=out_d[i0 : i0 + 128, :], in_=o_sb)

    nc.compile()
    return nc


def _get_built():
    global _built
    if _built is None:
        _built = _build()
    return _built


def _make_in_maps(query, key, value, Wq, bq, Wk, bk, Wv, bv, Wo, bo):
    bf = ml_dtypes.bfloat16
    xt = {}
    for b in range(B):
        xt[b] = {
            "xq_t": np.ascontiguousarray(query[b].T).astype(bf),
            "xk_t": np.ascontiguousarray(key[b].T).astype(bf),
            "xv_t": np.ascontiguousarray(value[b].T).astype(bf),
        }
    in_maps = []
    for c in range(NCORES):
        b, g = c // GROUPS, c % GROUPS
        cols = slice(g * CG, (g + 1) * CG)
        in_maps.append(
            {
                **xt[b],
                "wq": np.ascontiguousarray(Wq[:, cols]).astype(bf),
                "wk": np.ascontiguousarray(Wk[:, cols]).astype(bf),
                "wv": np.ascontiguousarray(Wv[:, cols]).astype(bf),
                "wo": np.ascontiguousarray(Wo[cols, :], dtype=np.float32),
                "bq": np.ascontiguousarray(bq[cols], dtype=np.float32),
                "bk": np.ascontiguousarray(bk[cols], dtype=np.float32),
            }
        )
    return in_maps


def kernel(query, key, value, Wq, bq, Wk, bk, Wv, bv, Wo, bo):
    global _last_results
    query = np.asarray(query, dtype=np.float32)
    key = np.asarray(key, dtype=np.float32)
    value = np.asarray(value, dtype=np.float32)
    Wq, Wk, Wv, Wo = (np.asarray(w, dtype=np.float32) for w in (Wq, Wk, Wv, Wo))
    bq, bk, bv, bo = (np.asarray(v, dtype=np.float32) for v in (bq, bk, bv, bo))

    nc = _get_built()
    in_maps = _make_in_maps(query, key, value, Wq, bq, Wk, bk, Wv, bv, Wo, bo)
    res = run_bass_kernel_spmd(nc, in_maps, core_ids=list(range(NCORES)))
    _last_results = res

    # bv contributes exactly bv @ Wo to every output row (softmax rows sum
    # to 1); bo is the plain output bias.
    bias = (bv @ Wo + bo).astype(np.float32)
    out = np.empty((B, L, D), dtype=np.float32)
    for b in range(B):
        acc = np.zeros((L, D), dtype=np.float32)
        for g in range(GROUPS):
            acc += res.results[b * GROUPS + g]["out_p"]
        out[b] = acc + bias
    return out


# revision 26
# speedup vs baseline: 1.1277x; 1.0494x over previous
"""Multi-head attention kernel for 8 Trainium2 NeuronCores (Bass/Tile).

Problem: B=2, L=2048, D=1024, H=16 heads, DK=64.
Sharding: core c -> batch b = c//4, head-group g = c%4 (4 heads each).
Each core computes its 4 heads' attention + its slice of the output
projection; the host sums the 4 partial outputs per batch (exact, since
Out = sum_g C_g @ Wo_g) and adds the bo / bv-derived bias terms.

Layout strategy (everything flows transposed so no on-chip transposes
are ever needed):
  - host supplies X^T [D, L] per input (bf16)
  - Q^T, K^T [256, L] produced directly (lhsT = W natural, rhs = X^T)
  - S^T[j,i] tiles via lhsT=K^T, rhs=Q^T; two heads packed per matmul
    round via PE row-tiling (K=64 -> row positions 0 and 64)
  - P~ = exp(S^T * scale) on ACT, PSUM -> SBUF (no max subtraction:
    |S*scale| <= ~3 for these input stats)
  - C~^T[dk,i] += V[j,dk].T-free matmul with a concurrent M=1 ones
    matmul (col position 64) accumulating the softmax denominators
  - normalization after the fact: recip(sums) broadcast across 64
    partitions with a K=1 matmul, then one DVE multiply
  - O[i,:] via lhsT=C^T, rhs=Wo natural; DMA out per 128-row tile
"""

import sys

sys.path.insert(0, "/opt/trn_rl_repo")

from contextlib import ExitStack

import ml_dtypes
import numpy as np

import concourse.bass as bass
import concourse.tile as tile
from concourse import bacc, mybir
from concourse.bass_utils import run_bass_kernel_spmd


def _install_ntff_hook_shim():
    """The agent image's ``antenv`` lacks ``axon_hooks``, so the boot shim
    silently skips NTFF-profile-hook registration and ``run_bass_kernel_spmd``
    crashes on import when BASS_TRACE=1. Provide the module and register the
    ctypes hook ourselves; degrade to no-tracing on any failure."""
    import types

    if "antenv.axon_hooks" in sys.modules:
        return
    mod = types.ModuleType("antenv.axon_hooks")
    mod._hook = None
    mod.set_axon_ntff_profile_hook = lambda h: setattr(mod, "_hook", h)
    mod.get_axon_ntff_profile_hook = lambda: mod._hook
    sys.modules["antenv.axon_hooks"] = mod
    try:
        import antenv

        antenv.axon_hooks = mod
    except Exception:
        pass
    try:
        from trn_agent_boot.trn_boot import _ntff_profile_via_ctypes

        mod._hook = _ntff_profile_via_ctypes("/opt/axon/libaxon_pjrt.so")
    except Exception:
        pass


_install_ntff_hook_shim()

B, L, D, H, DK = 2, 2048, 1024, 16, 64
NCORES = 8
GROUPS = 4  # head-groups == cores per batch
NH = H // GROUPS  # 4 heads per core
CG = NH * DK  # 256 projected features per core
DT = D // 128  # 8 contraction tiles
CT = CG // 128  # 2 c-tiles
IT = L // 512  # 4 query blocks of 512
LT = L // 128  # 16 key/query tiles of 128
SCALE = 1.0 / float(np.sqrt(DK))

F32 = mybir.dt.float32
BF16 = mybir.dt.bfloat16
Identity = mybir.ActivationFunctionType.Identity
Exp = mybir.ActivationFunctionType.Exp

_built = None
_last_results = None


import os

_DEBUG = bool(os.environ.get("KERNEL_DEBUG"))

if os.environ.get("KERNEL_LDW_OPT"):
    # A/B experiment: let walrus run its LDWEIGHTS optimization.
    from concourse import bass_utils as _bu

    _orig_run_command = _bu.run_command

    def _patched_run_command(cmd, *a, **kw):
        cmd = [
            c.replace("--enable-ldw-opt=false", "--enable-ldw-opt=true")
            if isinstance(c, str)
            else c
            for c in cmd
        ]
        return _orig_run_command(cmd, *a, **kw)

    _bu.run_command = _patched_run_command


def _build():
    nc = bacc.Bacc()

    xq_d = nc.dram_tensor("xq_t", [D, L], BF16, kind="ExternalInput")
    xk_d = nc.dram_tensor("xk_t", [D, L], BF16, kind="ExternalInput")
    xv_d = nc.dram_tensor("xv_t", [D, L], BF16, kind="ExternalInput")
    wq_d = nc.dram_tensor("wq", [D, CG], BF16, kind="ExternalInput")
    wk_d = nc.dram_tensor("wk", [D, CG], BF16, kind="ExternalInput")
    wv_d = nc.dram_tensor("wv", [D, CG], BF16, kind="ExternalInput")
    wo_d = nc.dram_tensor("wo", [CG, D], F32, kind="ExternalInput")
    bq_d = nc.dram_tensor("bq", [CG], F32, kind="ExternalInput")
    bk_d = nc.dram_tensor("bk", [CG], F32, kind="ExternalInput")
    out_d = nc.dram_tensor("out_p", [L, D], F32, kind="ExternalOutput")
    if _DEBUG:
        dbg = {
            "qT": nc.dram_tensor("d_qT", [CT, 128, L], F32, kind="ExternalOutput"),
            "kT": nc.dram_tensor("d_kT", [CT, 128, L], F32, kind="ExternalOutput"),
            "v": nc.dram_tensor("d_v", [128, LT * NH * 65], F32, kind="ExternalOutput"),
            "craw": nc.dram_tensor("d_craw", [CT, 128, L], F32, kind="ExternalOutput"),
            "sums": nc.dram_tensor("d_sums", [IT, 2048], F32, kind="ExternalOutput"),
            "strip": nc.dram_tensor("d_strip", [97, L], F32, kind="ExternalOutput"),
            "cnorm": nc.dram_tensor("d_cnorm", [CT, 128, L], F32, kind="ExternalOutput"),
        }

    with ExitStack() as ctx:
        tc = ctx.enter_context(tile.TileContext(nc))
        const = ctx.enter_context(tc.tile_pool(name="const", bufs=1))
        xp = ctx.enter_context(tc.tile_pool(name="xp", bufs=2))
        wp = ctx.enter_context(tc.tile_pool(name="wp", bufs=1))
        proj = ctx.enter_context(tc.tile_pool(name="proj", bufs=1))
        pp = ctx.enter_context(tc.tile_pool(name="pp", bufs=3))
        op_ = ctx.enter_context(tc.tile_pool(name="op", bufs=2))
        st = ctx.enter_context(tc.tile_pool(name="st", bufs=2))
        ph1 = ExitStack()
        ps1 = ph1.enter_context(tc.tile_pool(name="ps1", bufs=2, space="PSUM"))

        ones64 = const.tile([128, 64], F32)
        nc.vector.memset(ones64, 1.0)

        # ---------------- weights + biases ----------------
        wq_sb = wp.tile([128, DT, CG], BF16, tag="wq")
        wk_sb = wp.tile([128, DT, CG], BF16, tag="wk")
        wv_sb = wp.tile([128, DT, CG], BF16, tag="wv")
        wo_sb = wp.tile([128, CT, D], F32, tag="wo")
        nc.sync.dma_start(out=wq_sb, in_=wq_d[:, :].rearrange("(t p) c -> p t c", p=128))
        nc.sync.dma_start(out=wk_sb, in_=wk_d[:, :].rearrange("(t p) c -> p t c", p=128))
        nc.sync.dma_start(out=wv_sb, in_=wv_d[:, :].rearrange("(t p) c -> p t c", p=128))
        nc.sync.dma_start(out=wo_sb, in_=wo_d[:, :].rearrange("(t p) d -> p t d", p=128))
        bq_sb = wp.tile([128, CT], F32, tag="bq")
        bk_sb = wp.tile([128, CT], F32, tag="bk")
        nc.sync.dma_start(out=bq_sb, in_=bq_d[:].rearrange("(c p) -> p c", p=128))
        nc.sync.dma_start(out=bk_sb, in_=bk_d[:].rearrange("(c p) -> p c", p=128))

        # ---------------- phase 1: projections ----------------
        # V first (attention's C-matmuls need it), then K^T, then Q^T.
        with nc.named_scope("proj_v"):
            xv_sb = xp.tile([128, DT, L], BF16, tag="x", name="xv_sb")
            nc.sync.dma_start(
                out=xv_sb, in_=xv_d[:, :].rearrange("(t p) l -> p t l", p=128)
            )
            # v_sb holds [V_h | 1] blocks of 65 columns per head: the ones
            # column makes the C~ matmul also accumulate the softmax
            # denominator in psum row 64 (M=65 costs the same as M=64).
            v_sb = proj.tile([128, LT, NH * 65], F32, tag="v")
            nc.vector.memset(v_sb, 1.0)
            for lt in range(LT):
                v_ps = ps1.tile([128, CG], F32, tag="vps")
                for dt in range(DT):
                    nc.tensor.matmul(
                        v_ps,
                        lhsT=xv_sb[:, dt, lt * 128 : (lt + 1) * 128],
                        rhs=wv_sb[:, dt, :],
                        start=(dt == 0),
                        stop=(dt == DT - 1),
                    )
                # bv is folded into the host-side output bias (exact for
                # normalized softmax), so a plain copy suffices here.
                for h in range(NH):
                    nc.vector.tensor_copy(
                        out=v_sb[:, lt, 65 * h : 65 * h + 64],
                        in_=v_ps[:, 64 * h : 64 * h + 64],
                    )

        def project_T(x_sb, w_sb, b_sb, out_tiles, scope):
            with nc.named_scope(scope):
                for ct in range(CT):
                    for it in range(IT):
                        t_ps = ps1.tile([128, 512], F32, tag="tps", name="t_ps")
                        for dt in range(DT):
                            nc.tensor.matmul(
                                t_ps,
                                lhsT=w_sb[:, dt, ct * 128 : (ct + 1) * 128],
                                rhs=x_sb[:, dt, it * 512 : (it + 1) * 512],
                                start=(dt == 0),
                                stop=(dt == DT - 1),
                            )
                        nc.scalar.activation(
                            out=out_tiles[ct][:, it * 512 : (it + 1) * 512],
                            in_=t_ps,
                            func=Identity,
                            bias=b_sb[:, ct : ct + 1],
                            scale=1.0,
                        )

        xk_sb = xp.tile([128, DT, L], BF16, tag="x", name="xk_sb")
        nc.sync.dma_start(out=xk_sb, in_=xk_d[:, :].rearrange("(t p) l -> p t l", p=128))
        kT = [proj.tile([128, L], F32, tag=f"kT{ct}", name=f"kT{ct}") for ct in range(CT)]
        project_T(xk_sb, wk_sb, bk_sb, kT, "proj_k")

        xq_sb = xp.tile([128, DT, L], BF16, tag="x", name="xq_sb")
        nc.sync.dma_start(out=xq_sb, in_=xq_d[:, :].rearrange("(t p) l -> p t l", p=128))
        qT = [proj.tile([128, L], F32, tag=f"qT{ct}", name=f"qT{ct}") for ct in range(CT)]
        project_T(xq_sb, wq_sb, bq_sb, qT, "proj_q")

        if _DEBUG:
            for ct in range(CT):
                nc.sync.dma_start(out=dbg["qT"][ct], in_=qT[ct])
                nc.sync.dma_start(out=dbg["kT"][ct], in_=kT[ct])
            nc.sync.dma_start(
                out=dbg["v"][:, :], in_=v_sb[:, :, :].rearrange("p a b -> p (a b)")
            )

        # ---------------- phase 2+3: attention, streamed per i-block ----------------
        ph1.close()  # release phase-1 PSUM banks
        pss = ctx.enter_context(tc.tile_pool(name="pss", bufs=2, space="PSUM"))
        psc = ctx.enter_context(tc.tile_pool(name="psc", bufs=2, space="PSUM"))
        psn = ctx.enter_context(tc.tile_pool(name="psn", bufs=1, space="PSUM"))
        pso = ctx.enter_context(tc.tile_pool(name="pso", bufs=1, space="PSUM"))
        cT = [proj.tile([128, L], F32, tag=f"cT{ct}", name=f"cT{ct}") for ct in range(CT)]
        strip = st.tile([97, L], F32, tag="strip", bufs=1)

        def emit_attn(it, stage):
            """Attention for i-block `it`. The C~ matmuls are software-
            pipelined one jt behind the S matmuls: the PE queue is a strict
            in-order FIFO, so a C matmul waiting on exp(jt) must not sit in
            front of the already-ready S matmuls of jt+1."""
            isl = slice(it * 512, (it + 1) * 512)
            with nc.named_scope(f"attn_i{it}"):
                for hp in range(2):
                    cps = [
                        psc.tile([65, 512], F32, tag="cps", name=f"cps{hl}")
                        for hl in range(2)
                    ]

                    def emit_c(jt, p_t):
                        for hl in range(2):
                            h = 2 * hp + hl
                            psl = slice(hl * 512, (hl + 1) * 512)
                            nc.tensor.matmul(
                                cps[hl],
                                lhsT=v_sb[:, jt, 65 * h : 65 * h + 65],
                                rhs=p_t[:, psl],
                                start=(jt == 0),
                                stop=(jt == LT - 1),
                            )

                    pending = None  # (jt, p_t) with exp issued, C not yet
                    for jt in range(LT):
                        s_ps = pss.tile([128, 1024], F32, tag="sps", name="s_ps")
                        for hl in range(2):
                            # heads 2*hp+hl; rows 64*hl of the hp-th tile
                            rsl = slice(64 * hl, 64 * hl + 64)
                            nc.tensor.matmul(
                                s_ps[:, hl * 512 : (hl + 1) * 512],
                                lhsT=kT[hp][rsl, jt * 128 : (jt + 1) * 128],
                                rhs=qT[hp][rsl, isl],
                                start=True,
                                stop=True,
                            )
                        p_t = pp.tile([128, 1024], F32, tag="pt", name="p_t")
                        nc.scalar.activation(out=p_t, in_=s_ps, func=Exp, scale=SCALE)
                        if pending is not None:
                            emit_c(*pending)
                        pending = (jt, p_t)
                    emit_c(*pending)
                    for hl in range(2):
                        h = 2 * hp + hl
                        nc.vector.tensor_copy(
                            out=cT[hp][64 * hl : 64 * hl + 64, isl],
                            in_=cps[hl][0:64, :],
                        )
                        nc.vector.tensor_copy(
                            out=stage[64:65, h * 512 : (h + 1) * 512],
                            in_=cps[hl][64:65, :],
                        )

        def emit_norm_o(it, stage):
            isl = slice(it * 512, (it + 1) * 512)
            if _DEBUG:
                for ct in range(CT):
                    nc.sync.dma_start(out=dbg["craw"][ct][:, isl], in_=cT[ct][:, isl])
                nc.sync.dma_start(
                    out=dbg["sums"][it : it + 1, :], in_=stage[64:65, :]
                )

            with nc.named_scope(f"norm_o_i{it}"):
                # Reciprocal of the four heads' denominators. DVE reciprocal
                # is ~8 cyc/elem and partition-parallel, so spread the 2048
                # sums across 128 partitions with a tiny SBUF->SBUF DMA first
                # (a single-row reciprocal would serialize ~17us on one lane).
                sq = st.tile([128, 16], F32, tag="sq")
                sq2 = st.tile([128, 16], F32, tag="sq2")
                nc.sync.dma_start(out=sq[:, :], in_=stage[64:65, :])
                nc.vector.reciprocal(out=sq2, in_=sq)
                for h in range(NH):
                    # sq2 partitions 32h..32h+31 hold head h's 512 values
                    nc.sync.dma_start(
                        out=strip[32 * h : 32 * h + 1, isl],
                        in_=sq2[32 * h : 32 * h + 32, :],
                    )
                for hp in range(2):
                    n_ps = psn.tile([128, 512], F32, tag="nps")
                    for hl in range(2):
                        h = 2 * hp + hl
                        nc.tensor.matmul(
                            n_ps[64 * hl : 64 * hl + 64, :],
                            lhsT=ones64[32 * h : 32 * h + 1, :],
                            rhs=strip[32 * h : 32 * h + 1, isl],
                            start=True,
                            stop=True,
                            tile_position=(32 * h, 64 * hl),
                        )
                    for hl in range(2):
                        rsl = slice(64 * hl, 64 * hl + 64)
                        nc.vector.tensor_mul(
                            out=cT[hp][rsl, isl],
                            in0=cT[hp][rsl, isl],
                            in1=n_ps[rsl, :],
                        )
                if _DEBUG:
                    for ct in range(CT):
                        nc.sync.dma_start(
                            out=dbg["cnorm"][ct][:, isl], in_=cT[ct][:, isl]
                        )
                    if it == IT - 1:
                        nc.sync.dma_start(out=dbg["strip"][:, :], in_=strip)
                for s in range(4):
                    i0 = it * 512 + s * 128
                    o_sb = op_.tile([128, D], F32, tag="osb")
                    for dn in range(2):
                        o_ps = pso.tile([128, 512], F32, tag="ops")
                        for ct in range(CT):
                            nc.tensor.matmul(
                                o_ps,
                                lhsT=cT[ct][:, i0 : i0 + 128],
                                rhs=wo_sb[:, ct, dn * 512 : (dn + 1) * 512],
                                start=(ct == 0),
                                stop=(ct == CT - 1),
                            )
                        nc.vector.tensor_copy(
                            out=o_sb[:, dn * 512 : (dn + 1) * 512], in_=o_ps
                        )
                    nc.sync.dma_start(out=out_d[i0 : i0 + 128, :], in_=o_sb)

        # norm+O for block it-1 is emitted after attention for block it: its
        # PE instructions depend on a DMA->reciprocal->DMA chain, and the
        # in-order PE queue would otherwise stall the next block behind it.
        prev_stage = None
        for it in range(IT):
            stage = st.tile([65, 2048], F32, tag="stage")
            emit_attn(it, stage)
            if prev_stage is not None:
                emit_norm_o(it - 1, prev_stage)
            prev_stage = stage
        emit_norm_o(IT - 1, prev_stage)

    nc.compile()
    return nc


def _get_built():
    global _built
    if _built is None:
        _built = _build()
    return _built


def _make_in_maps(query, key, value, Wq, bq, Wk, bk, Wv, bv, Wo, bo):
    bf = ml_dtypes.bfloat16
    xt = {}
    for b in range(B):
        xt[b] = {
            "xq_t": np.ascontiguousarray(query[b].T).astype(bf),
            "xk_t": np.ascontiguousarray(key[b].T).astype(bf),
            "xv_t": np.ascontiguousarray(value[b].T).astype(bf),
        }
    in_maps = []
    for c in range(NCORES):
        b, g = c // GROUPS, c % GROUPS
        cols = slice(g * CG, (g + 1) * CG)
        in_maps.append(
            {
                **xt[b],
                "wq": np.ascontiguousarray(Wq[:, cols]).astype(bf),
                "wk": np.ascontiguousarray(Wk[:, cols]).astype(bf),
                "wv": np.ascontiguousarray(Wv[:, cols]).astype(bf),
                "wo": np.ascontiguousarray(Wo[cols, :], dtype=np.float32),
                "bq": np.ascontiguousarray(bq[cols], dtype=np.float32),
                "bk": np.ascontiguousarray(bk[cols], dtype=np.float32),
            }
        )
    return in_maps


def kernel(query, key, value, Wq, bq, Wk, bk, Wv, bv, Wo, bo):
    global _last_results
    query = np.asarray(query, dtype=np.float32)
    key = np.asarray(key, dtype=np.float32)
    value = np.asarray(value, dtype=np.float32)
    Wq, Wk, Wv, Wo = (np.asarray(w, dtype=np.float32) for w in (Wq, Wk, Wv, Wo))
    bq, bk, bv, bo = (np.asarray(v, dtype=np.float32) for v in (bq, bk, bv, bo))

    nc = _get_built()
    in_maps = _make_in_maps(query, key, value, Wq, bq, Wk, bk, Wv, bv, Wo, bo)
    res = run_bass_kernel_spmd(nc, in_maps, core_ids=list(range(NCORES)))
    _last_results = res

    # bv contributes exactly bv @ Wo to every output row (softmax rows sum
    # to 1); bo is the plain output bias.
    bias = (bv @ Wo + bo).astype(np.float32)
    out = np.empty((B, L, D), dtype=np.float32)
    for b in range(B):
        acc = np.zeros((L, D), dtype=np.float32)
        for g in range(GROUPS):
            acc += res.results[b * GROUPS + g]["out_p"]
        out[b] = acc + bias
    return out


# revision 28
# speedup vs baseline: 2.3004x; 2.0399x over previous
"""Multi-head attention kernel for 8 Trainium2 NeuronCores (Bass/Tile).

Problem: B=2, L=2048, D=1024, H=16 heads, DK=64.
Sharding: core c -> batch b = c//4, head-group g = c%4 (4 heads each).
Each core computes its 4 heads' attention + its slice of the output
projection; the host sums the 4 partial outputs per batch (exact, since
Out = sum_g C_g @ Wo_g) and adds the bo / bv-derived bias terms.

Layout strategy (everything flows transposed so no on-chip transposes
are ever needed):
  - host supplies X^T [D, L] per input (bf16)
  - Q^T, K^T [256, L] produced directly (lhsT = W natural, rhs = X^T)
  - S^T[j,i] tiles via lhsT=K^T, rhs=Q^T; two heads packed per matmul
    round via PE row-tiling (K=64 -> row positions 0 and 64)
  - P~ = exp(S^T * scale) on ACT, PSUM -> SBUF (no max subtraction:
    |S*scale| <= ~3 for these input stats)
  - C~^T[dk,i] += V[j,dk].T-free matmul with a concurrent M=1 ones
    matmul (col position 64) accumulating the softmax denominators
  - normalization after the fact: recip(sums) broadcast across 64
    partitions with a K=1 matmul, then one DVE multiply
  - O[i,:] via lhsT=C^T, rhs=Wo natural; DMA out per 128-row tile
"""

import sys

sys.path.insert(0, "/opt/trn_rl_repo")

from contextlib import ExitStack

import ml_dtypes
import numpy as np

import concourse.bass as bass
import concourse.tile as tile
from concourse import bacc, mybir
from concourse.bass_utils import run_bass_kernel_spmd


def _install_ntff_hook_shim():
    """The agent image's ``antenv`` lacks ``axon_hooks``, so the boot shim
    silently skips NTFF-profile-hook registration and ``run_bass_kernel_spmd``
    crashes on import when BASS_TRACE=1. Provide the module and register the
    ctypes hook ourselves; degrade to no-tracing on any failure."""
    import types

    if "antenv.axon_hooks" in sys.modules:
        return
    mod = types.ModuleType("antenv.axon_hooks")
    mod._hook = None
    mod.set_axon_ntff_profile_hook = lambda h: setattr(mod, "_hook", h)
    mod.get_axon_ntff_profile_hook = lambda: mod._hook
    sys.modules["antenv.axon_hooks"] = mod
    try:
        import antenv

        antenv.axon_hooks = mod
    except Exception:
        pass
    try:
        from trn_agent_boot.trn_boot import _ntff_profile_via_ctypes

        mod._hook = _ntff_profile_via_ctypes("/opt/axon/libaxon_pjrt.so")
    except Exception:
        pass


_install_ntff_hook_shim()

B, L, D, H, DK = 2, 2048, 1024, 16, 64
NCORES = 8
GROUPS = 4  # head-groups == cores per batch
NH = H // GROUPS  # 4 heads per core
CG = NH * DK  # 256 projected features per core
DT = D // 128  # 8 contraction tiles
CT = CG // 128  # 2 c-tiles
IT = L // 512  # 4 query blocks of 512
LT = L // 128  # 16 key/query tiles of 128
SCALE = 1.0 / float(np.sqrt(DK))

F32 = mybir.dt.float32
BF16 = mybir.dt.bfloat16
Identity = mybir.ActivationFunctionType.Identity
Exp = mybir.ActivationFunctionType.Exp

_built = None
_last_results = None


import os

_DEBUG = bool(os.environ.get("KERNEL_DEBUG"))

if os.environ.get("KERNEL_LDW_OPT"):
    # A/B experiment: let walrus run its LDWEIGHTS optimization.
    from concourse import bass_utils as _bu

    _orig_run_command = _bu.run_command

    def _patched_run_command(cmd, *a, **kw):
        cmd = [
            c.replace("--enable-ldw-opt=false", "--enable-ldw-opt=true")
            if isinstance(c, str)
            else c
            for c in cmd
        ]
        return _orig_run_command(cmd, *a, **kw)

    _bu.run_command = _patched_run_command


def _build():
    nc = bacc.Bacc()

    xq_d = nc.dram_tensor("xq_t", [D, L], BF16, kind="ExternalInput")
    xk_d = nc.dram_tensor("xk_t", [D, L], BF16, kind="ExternalInput")
    xv_d = nc.dram_tensor("xv_t", [D, L], BF16, kind="ExternalInput")
    wq_d = nc.dram_tensor("wq", [D, CG], BF16, kind="ExternalInput")
    wk_d = nc.dram_tensor("wk", [D, CG], BF16, kind="ExternalInput")
    wv_d = nc.dram_tensor("wv", [D, CG], BF16, kind="ExternalInput")
    wo_d = nc.dram_tensor("wo", [CG, D], BF16, kind="ExternalInput")
    bq_d = nc.dram_tensor("bq", [CG], F32, kind="ExternalInput")
    bk_d = nc.dram_tensor("bk", [CG], F32, kind="ExternalInput")
    out_d = nc.dram_tensor("out_p", [L, D], F32, kind="ExternalOutput")
    if _DEBUG:
        dbg = {
            "qT": nc.dram_tensor("d_qT", [CT, 128, L], BF16, kind="ExternalOutput"),
            "kT": nc.dram_tensor("d_kT", [CT, 128, L], BF16, kind="ExternalOutput"),
            "v": nc.dram_tensor("d_v", [128, LT * NH * 65], BF16, kind="ExternalOutput"),
            "craw": nc.dram_tensor("d_craw", [CT, 128, L], BF16, kind="ExternalOutput"),
            "sums": nc.dram_tensor("d_sums", [IT, 2048], F32, kind="ExternalOutput"),
            "strip": nc.dram_tensor("d_strip", [97, L], F32, kind="ExternalOutput"),
            "cnorm": nc.dram_tensor("d_cnorm", [CT, 128, L], BF16, kind="ExternalOutput"),
        }

    with ExitStack() as ctx:
        tc = ctx.enter_context(tile.TileContext(nc))
        const = ctx.enter_context(tc.tile_pool(name="const", bufs=1))
        xp = ctx.enter_context(tc.tile_pool(name="xp", bufs=2))
        wp = ctx.enter_context(tc.tile_pool(name="wp", bufs=1))
        proj = ctx.enter_context(tc.tile_pool(name="proj", bufs=1))
        pp = ctx.enter_context(tc.tile_pool(name="pp", bufs=3))
        op_ = ctx.enter_context(tc.tile_pool(name="op", bufs=2))
        st = ctx.enter_context(tc.tile_pool(name="st", bufs=2))
        ph1 = ExitStack()
        ps1 = ph1.enter_context(tc.tile_pool(name="ps1", bufs=2, space="PSUM"))

        ones64 = const.tile([128, 64], F32)
        nc.vector.memset(ones64, 1.0)

        # ---------------- weights + biases ----------------
        wq_sb = wp.tile([128, DT, CG], BF16, tag="wq")
        wk_sb = wp.tile([128, DT, CG], BF16, tag="wk")
        wv_sb = wp.tile([128, DT, CG], BF16, tag="wv")
        wo_sb = wp.tile([128, CT, D], BF16, tag="wo")
        nc.sync.dma_start(out=wq_sb, in_=wq_d[:, :].rearrange("(t p) c -> p t c", p=128))
        nc.sync.dma_start(out=wk_sb, in_=wk_d[:, :].rearrange("(t p) c -> p t c", p=128))
        nc.sync.dma_start(out=wv_sb, in_=wv_d[:, :].rearrange("(t p) c -> p t c", p=128))
        nc.sync.dma_start(out=wo_sb, in_=wo_d[:, :].rearrange("(t p) d -> p t d", p=128))
        bq_sb = wp.tile([128, CT], F32, tag="bq")
        bk_sb = wp.tile([128, CT], F32, tag="bk")
        nc.sync.dma_start(out=bq_sb, in_=bq_d[:].rearrange("(c p) -> p c", p=128))
        nc.sync.dma_start(out=bk_sb, in_=bk_d[:].rearrange("(c p) -> p c", p=128))

        # ---------------- phase 1: projections ----------------
        # V first (attention's C-matmuls need it), then K^T, then Q^T.
        with nc.named_scope("proj_v"):
            xv_sb = xp.tile([128, DT, L], BF16, tag="x", name="xv_sb")
            nc.sync.dma_start(
                out=xv_sb, in_=xv_d[:, :].rearrange("(t p) l -> p t l", p=128)
            )
            # v_sb holds [V_h | 1] blocks of 65 columns per head: the ones
            # column makes the C~ matmul also accumulate the softmax
            # denominator in psum row 64 (M=65 costs the same as M=64).
            v_sb = proj.tile([128, LT, NH * 65], BF16, tag="v")
            nc.vector.memset(v_sb, 1.0)
            for lt in range(LT):
                v_ps = ps1.tile([128, CG], F32, tag="vps")
                for dt in range(DT):
                    nc.tensor.matmul(
                        v_ps,
                        lhsT=xv_sb[:, dt, lt * 128 : (lt + 1) * 128],
                        rhs=wv_sb[:, dt, :],
                        start=(dt == 0),
                        stop=(dt == DT - 1),
                    )
                # bv is folded into the host-side output bias (exact for
                # normalized softmax), so a plain copy suffices here.
                for h in range(NH):
                    nc.vector.tensor_copy(
                        out=v_sb[:, lt, 65 * h : 65 * h + 64],
                        in_=v_ps[:, 64 * h : 64 * h + 64],
                    )

        def project_T(x_sb, w_sb, b_sb, out_tiles, scope):
            with nc.named_scope(scope):
                for ct in range(CT):
                    for it in range(IT):
                        t_ps = ps1.tile([128, 512], F32, tag="tps", name="t_ps")
                        for dt in range(DT):
                            nc.tensor.matmul(
                                t_ps,
                                lhsT=w_sb[:, dt, ct * 128 : (ct + 1) * 128],
                                rhs=x_sb[:, dt, it * 512 : (it + 1) * 512],
                                start=(dt == 0),
                                stop=(dt == DT - 1),
                            )
                        nc.scalar.activation(
                            out=out_tiles[ct][:, it * 512 : (it + 1) * 512],
                            in_=t_ps,
                            func=Identity,
                            bias=b_sb[:, ct : ct + 1],
                            scale=1.0,
                        )

        xk_sb = xp.tile([128, DT, L], BF16, tag="x", name="xk_sb")
        nc.sync.dma_start(out=xk_sb, in_=xk_d[:, :].rearrange("(t p) l -> p t l", p=128))
        kT = [proj.tile([128, L], BF16, tag=f"kT{ct}", name=f"kT{ct}") for ct in range(CT)]
        project_T(xk_sb, wk_sb, bk_sb, kT, "proj_k")

        xq_sb = xp.tile([128, DT, L], BF16, tag="x", name="xq_sb")
        nc.sync.dma_start(out=xq_sb, in_=xq_d[:, :].rearrange("(t p) l -> p t l", p=128))
        qT = [proj.tile([128, L], BF16, tag=f"qT{ct}", name=f"qT{ct}") for ct in range(CT)]
        project_T(xq_sb, wq_sb, bq_sb, qT, "proj_q")

        if _DEBUG:
            for ct in range(CT):
                nc.sync.dma_start(out=dbg["qT"][ct], in_=qT[ct])
                nc.sync.dma_start(out=dbg["kT"][ct], in_=kT[ct])
            nc.sync.dma_start(
                out=dbg["v"][:, :], in_=v_sb[:, :, :].rearrange("p a b -> p (a b)")
            )

        # ---------------- phase 2+3: attention, streamed per i-block ----------------
        ph1.close()  # release phase-1 PSUM banks
        pss = ctx.enter_context(tc.tile_pool(name="pss", bufs=2, space="PSUM"))
        psc = ctx.enter_context(tc.tile_pool(name="psc", bufs=2, space="PSUM"))
        psn = ctx.enter_context(tc.tile_pool(name="psn", bufs=1, space="PSUM"))
        pso = ctx.enter_context(tc.tile_pool(name="pso", bufs=1, space="PSUM"))
        cT = [proj.tile([128, L], BF16, tag=f"cT{ct}", name=f"cT{ct}") for ct in range(CT)]
        strip = st.tile([97, L], F32, tag="strip", bufs=1)

        def emit_attn(it, stage):
            """Attention for i-block `it`. The C~ matmuls are software-
            pipelined one jt behind the S matmuls: the PE queue is a strict
            in-order FIFO, so a C matmul waiting on exp(jt) must not sit in
            front of the already-ready S matmuls of jt+1."""
            isl = slice(it * 512, (it + 1) * 512)
            with nc.named_scope(f"attn_i{it}"):
                for hp in range(2):
                    cps = [
                        psc.tile([65, 512], F32, tag="cps", name=f"cps{hl}")
                        for hl in range(2)
                    ]

                    def emit_c(jt, p_t):
                        for hl in range(2):
                            h = 2 * hp + hl
                            psl = slice(hl * 512, (hl + 1) * 512)
                            nc.tensor.matmul(
                                cps[hl],
                                lhsT=v_sb[:, jt, 65 * h : 65 * h + 65],
                                rhs=p_t[:, psl],
                                start=(jt == 0),
                                stop=(jt == LT - 1),
                            )

                    pending = None  # (jt, p_t) with exp issued, C not yet
                    for jt in range(LT):
                        s_ps = pss.tile([128, 1024], F32, tag="sps", name="s_ps")
                        for hl in range(2):
                            # heads 2*hp+hl; rows 64*hl of the hp-th tile
                            rsl = slice(64 * hl, 64 * hl + 64)
                            nc.tensor.matmul(
                                s_ps[:, hl * 512 : (hl + 1) * 512],
                                lhsT=kT[hp][rsl, jt * 128 : (jt + 1) * 128],
                                rhs=qT[hp][rsl, isl],
                                start=True,
                                stop=True,
                            )
                        p_t = pp.tile([128, 1024], BF16, tag="pt", name="p_t")
                        nc.scalar.activation(out=p_t, in_=s_ps, func=Exp, scale=SCALE)
                        if pending is not None:
                            emit_c(*pending)
                        pending = (jt, p_t)
                    emit_c(*pending)
                    for hl in range(2):
                        h = 2 * hp + hl
                        nc.vector.tensor_copy(
                            out=cT[hp][64 * hl : 64 * hl + 64, isl],
                            in_=cps[hl][0:64, :],
                        )
                        nc.vector.tensor_copy(
                            out=stage[64:65, h * 512 : (h + 1) * 512],
                            in_=cps[hl][64:65, :],
                        )

        def emit_norm_o(it, stage):
            isl = slice(it * 512, (it + 1) * 512)
            if _DEBUG:
                for ct in range(CT):
                    nc.sync.dma_start(out=dbg["craw"][ct][:, isl], in_=cT[ct][:, isl])
                nc.sync.dma_start(
                    out=dbg["sums"][it : it + 1, :], in_=stage[64:65, :]
                )

            with nc.named_scope(f"norm_o_i{it}"):
                # Reciprocal of the four heads' denominators. DVE reciprocal
                # is ~8 cyc/elem and partition-parallel, so spread the 2048
                # sums across 128 partitions with a tiny SBUF->SBUF DMA first
                # (a single-row reciprocal would serialize ~17us on one lane).
                sq = st.tile([128, 16], F32, tag="sq")
                sq2 = st.tile([128, 16], F32, tag="sq2")
                nc.sync.dma_start(out=sq[:, :], in_=stage[64:65, :])
                nc.vector.reciprocal(out=sq2, in_=sq)
                for h in range(NH):
                    # sq2 partitions 32h..32h+31 hold head h's 512 values
                    nc.sync.dma_start(
                        out=strip[32 * h : 32 * h + 1, isl],
                        in_=sq2[32 * h : 32 * h + 32, :],
                    )
                for hp in range(2):
                    n_ps = psn.tile([128, 512], F32, tag="nps")
                    for hl in range(2):
                        h = 2 * hp + hl
                        nc.tensor.matmul(
                            n_ps[64 * hl : 64 * hl + 64, :],
                            lhsT=ones64[32 * h : 32 * h + 1, :],
                            rhs=strip[32 * h : 32 * h + 1, isl],
                            start=True,
                            stop=True,
                            tile_position=(32 * h, 64 * hl),
                        )
                    for hl in range(2):
                        rsl = slice(64 * hl, 64 * hl + 64)
                        nc.vector.tensor_mul(
                            out=cT[hp][rsl, isl],
                            in0=cT[hp][rsl, isl],
                            in1=n_ps[rsl, :],
                        )
                if _DEBUG:
                    for ct in range(CT):
                        nc.sync.dma_start(
                            out=dbg["cnorm"][ct][:, isl], in_=cT[ct][:, isl]
                        )
                    if it == IT - 1:
                        nc.sync.dma_start(out=dbg["strip"][:, :], in_=strip)
                for s in range(4):
                    i0 = it * 512 + s * 128
                    o_sb = op_.tile([128, D], F32, tag="osb")
                    for dn in range(2):
                        o_ps = pso.tile([128, 512], F32, tag="ops")
                        for ct in range(CT):
                            nc.tensor.matmul(
                                o_ps,
                                lhsT=cT[ct][:, i0 : i0 + 128],
                                rhs=wo_sb[:, ct, dn * 512 : (dn + 1) * 512],
                                start=(ct == 0),
                                stop=(ct == CT - 1),
                            )
                        nc.vector.tensor_copy(
                            out=o_sb[:, dn * 512 : (dn + 1) * 512], in_=o_ps
                        )
                    nc.sync.dma_start(out=out_d[i0 : i0 + 128, :], in_=o_sb)

        # norm+O for block it-1 is emitted after attention for block it: its
        # PE instructions depend on a DMA->reciprocal->DMA chain, and the
        # in-order PE queue would otherwise stall the next block behind it.
        prev_stage = None
        for it in range(IT):
            stage = st.tile([65, 2048], F32, tag="stage")
            emit_attn(it, stage)
            if prev_stage is not None:
                emit_norm_o(it - 1, prev_stage)
            prev_stage = stage
        emit_norm_o(IT - 1, prev_stage)

    nc.compile()
    return nc


def _get_built():
    global _built
    if _built is None:
        _built = _build()
    return _built


def _make_in_maps(query, key, value, Wq, bq, Wk, bk, Wv, bv, Wo, bo):
    bf = ml_dtypes.bfloat16
    xt = {}
    for b in range(B):
        xt[b] = {
            "xq_t": np.ascontiguousarray(query[b].T).astype(bf),
            "xk_t": np.ascontiguousarray(key[b].T).astype(bf),
            "xv_t": np.ascontiguousarray(value[b].T).astype(bf),
        }
    in_maps = []
    for c in range(NCORES):
        b, g = c // GROUPS, c % GROUPS
        cols = slice(g * CG, (g + 1) * CG)
        in_maps.append(
            {
                **xt[b],
                "wq": np.ascontiguousarray(Wq[:, cols]).astype(bf),
                "wk": np.ascontiguousarray(Wk[:, cols]).astype(bf),
                "wv": np.ascontiguousarray(Wv[:, cols]).astype(bf),
                "wo": np.ascontiguousarray(Wo[cols, :]).astype(bf),
                "bq": np.ascontiguousarray(bq[cols], dtype=np.float32),
                "bk": np.ascontiguousarray(bk[cols], dtype=np.float32),
            }
        )
    return in_maps


def kernel(query, key, value, Wq, bq, Wk, bk, Wv, bv, Wo, bo):
    global _last_results
    query = np.asarray(query, dtype=np.float32)
    key = np.asarray(key, dtype=np.float32)
    value = np.asarray(value, dtype=np.float32)
    Wq, Wk, Wv, Wo = (np.asarray(w, dtype=np.float32) for w in (Wq, Wk, Wv, Wo))
    bq, bk, bv, bo = (np.asarray(v, dtype=np.float32) for v in (bq, bk, bv, bo))

    nc = _get_built()
    in_maps = _make_in_maps(query, key, value, Wq, bq, Wk, bk, Wv, bv, Wo, bo)
    res = run_bass_kernel_spmd(nc, in_maps, core_ids=list(range(NCORES)))
    _last_results = res

    # bv contributes exactly bv @ Wo to every output row (softmax rows sum
    # to 1); bo is the plain output bias.
    bias = (bv @ Wo + bo).astype(np.float32)
    out = np.empty((B, L, D), dtype=np.float32)
    for b in range(B):
        acc = np.zeros((L, D), dtype=np.float32)
        for g in range(GROUPS):
            acc += res.results[b * GROUPS + g]["out_p"]
        out[b] = acc + bias
    return out
